# revision 1
# baseline (speedup 1.0000x reference)
"""Trainium2 Bass kernel for nn_Attention_67637144977803.

Dense transformer attention block (XCiT-style, L2-normalized q/k along the
token axis), B=2, C=256, H=W=48 (N=2304 tokens), 8 heads x 64 dims.

Sharding: the 16 (batch, head) pairs are sharded 2-per-core across the 8
NeuronCores (cores 0-3: batch 0, cores 4-7: batch 1; core c%4 owns heads
2*(c%4), 2*(c%4)+1). Each core:
  1. computes its q/k/v slices via the 1x1-conv matmul (weights pre-sliced
     and pre-transposed on the host),
  2. l2-normalizes q, k along tokens,
  3. computes attention in the transposed layout S^T[m, n] = sum_d k[d,m]q[d,n]
     so softmax's contraction dim (m) lands on PSUM partitions,
  4. exp on the scalar engine (no max subtraction: normalized q/k make
     |S| < ~0.1, so exp is safely in range),
  5. AV matmul with a ones-row appended to v^T, which makes the softmax
     denominator fall out as row 64 of the PSUM accumulator,
  6. divides via reciprocal + a DMA round-trip through DRAM that
     broadcasts the reciprocal row across partitions + multiply (the PE
     ones-matmul variant is used for the last item to shorten the tail),
  7. applies its slice of the output projection; the host sums the 4 partial
     projections per batch (bias is fed only to one core per batch).

All big matmuls run as float32r (full-rate fp32 on the PE); producers of
f32r-consumed data emit f32r so the BIR verifier's rounding rule holds
(DMA'd inputs are pre-rounded to f32r on the host).

The (block, head) work items are software-pipelined: item i's QK+exp is
emitted BEFORE item i-1's AV+divide, so the scalar engine (the bottleneck:
~10.6M exp elements per core) never starves while the PE drains the
previous item's AV accumulation and projection.
"""

import os
import sys

import numpy as np

for _p in ("/opt/trn_rl_repo", "/root/.axon_site/_ro/trn_rl_repo"):
    if os.path.isdir(_p) and _p not in sys.path:
        sys.path.insert(0, _p)

import concourse.bacc as bacc
import concourse.mybir as mybir
import concourse.tile as tile
from concourse import bass_utils

F32 = mybir.dt.float32
F32R = mybir.dt.float32r

B = 2
C = 256
N = 2304  # 48*48 tokens
N_HEADS = 8
D = 64  # head dim
HEADS_PER_CORE = 2
N_CORES = 8
M_TILES = N // 128  # 18 contraction tiles over tokens
EXP_GROUP = 3  # QK psum banks per exp instruction
# token blocks (start, width); PSUM bank = 512 f32
BLOCKS = [(0, 512), (512, 512), (1024, 512), (1536, 512), (2048, 256)]

_CACHE = {}


def _build_kernel():
    """Build the (single-program SPMD) Bass module."""
    nc = bacc.Bacc("TRN2", target_bir_lowering=False, debug=False)

    x_d = nc.dram_tensor("x", [C, N], F32R, kind="ExternalInput").ap()
    wq_d = nc.dram_tensor("wq", [C, 128], F32R, kind="ExternalInput").ap()
    wk_d = nc.dram_tensor("wk", [C, 128], F32R, kind="ExternalInput").ap()
    wv_d = nc.dram_tensor("wv", [C, 128], F32R, kind="ExternalInput").ap()
    wp_d = nc.dram_tensor("wp", [128, C], F32R, kind="ExternalInput").ap()
    ident_d = nc.dram_tensor("ident", [128, 128], F32, kind="ExternalInput").ap()
    ones_d = nc.dram_tensor("ones", [128, 64], F32R, kind="ExternalInput").ap()
    bias_d = nc.dram_tensor("bias", [C, 1], F32, kind="ExternalInput").ap()
    y_d = nc.dram_tensor("y", [C, N], F32, kind="ExternalOutput").ap()

    with tile.TileContext(nc) as tc:
        _kernel_body(tc, x_d, wq_d, wk_d, wv_d, wp_d, ident_d, ones_d, bias_d, y_d)

    nc.compile()
    return nc


def _kernel_body(tc, x_d, wq_d, wk_d, wv_d, wp_d, ident_d, ones_d, bias_d, y_d):
    nc = tc.nc
    Exp = mybir.ActivationFunctionType.Exp

    from contextlib import ExitStack

    ctx = ExitStack()
    with ctx:
        const_pool = ctx.enter_context(tc.tile_pool(name="const", bufs=1))
        xw_pool = ctx.enter_context(tc.tile_pool(name="xw", bufs=1))
        qkv_pool = ctx.enter_context(tc.tile_pool(name="qkv", bufs=1))
        sexp_pool = ctx.enter_context(tc.tile_pool(name="sexp", bufs=2))
        small_pool = ctx.enter_context(tc.tile_pool(name="small", bufs=2))
        dram_pool = ctx.enter_context(tc.tile_pool(name="dscr", bufs=4, space="DRAM"))
        psum_s = ctx.enter_context(tc.tile_pool(name="ps", bufs=2, space="PSUM"))
        psum_av = ctx.enter_context(tc.tile_pool(name="pav", bufs=2, space="PSUM"))

        # ---- DMA loads, critical-path first: x chunk 0, then wk (the first
        # qkv matmuls), then the rest. Host pre-rounds all f32r data, so the
        # f32r tensors are DMA'd directly with no staging copies.
        xv = x_d.rearrange("(a p) n -> p a n", p=128)
        x_sb = xw_pool.tile([128, 2, N], F32R, name="x_sb")
        w_sb = xw_pool.tile([128, 3, 2, 128], F32R, name="w_sb")
        ident_sb = const_pool.tile([128, 128], F32, name="ident_sb")
        nc.sync.dma_start(ident_sb[:], ident_d)
        for kk in range(2):
            nc.sync.dma_start(x_sb[:, kk, 0:1536], xv[:, kk, 0:1536])
        for wi, wd in ((0, wq_d), (1, wk_d), (2, wv_d)):
            nc.sync.dma_start(w_sb[:, wi], wd.rearrange("(a p) m -> p a m", p=128))
        for kk in range(2):
            nc.sync.dma_start(x_sb[:, kk, 1536:N], xv[:, kk, 1536:N])
        wp_sb = xw_pool.tile([128, C], F32R, name="wp_sb")
        nc.sync.dma_start(wp_sb[:], wp_d)
        ones_sb = const_pool.tile([128, 64], F32R, name="ones_sb")
        nc.sync.dma_start(ones_sb[:], ones_d)
        ones_col = ones_sb  # [:, 0:1] used for the vT ones column
        bias_sb = const_pool.tile([128, 2], F32, name="bias_sb")
        nc.sync.dma_start(bias_sb[:], bias_d.rearrange("(a p) one -> p (a one)", p=128))

        # ---- PE warm-up: ~4us of tiny f32 matmuls on the identity while the
        # big DMAs are in flight, so qkv starts at the full 2.4 GHz clock.
        for wu in range(6):
            wt = psum_av.tile([128, 512], F32, tag="av", name=f"warm_{wu}")
            nc.tensor.matmul(
                wt[:, 0:128], ident_sb[:], ident_sb[:], start=True, stop=True
            )

        # ---- qkv projection: [128 rows = 2 heads x 64, N]; k and q first
        # (the QK critical path), v last (transposes overlap the first exps).
        # Norm partial sums are computed per chunk to overlap the chain.
        q_sb = qkv_pool.tile([128, N], F32R, name="q_sb")
        k_sb = qkv_pool.tile([128, N], F32R, name="k_sb")
        v_sb = qkv_pool.tile([128, N], F32, name="v_sb")
        ss_parts = {}
        def emit_qkv(which):
            for wi, dst in which:
                _emit_qkv_one(wi, dst)

        def _emit_qkv_one(wi, dst):
            for ci, (base, wdt) in enumerate(((0, 1536), (1536, 768))):
                pt = psum_s.tile([128, 1536], F32, tag="ps",
                                 name=f"qkv_ps_{wi}_{base}")
                for j in range(0, wdt, 512):
                    w_ = min(512, wdt - j)
                    for kk in range(2):
                        nc.tensor.matmul(
                            pt[:, j : j + w_],
                            w_sb[:, wi, kk],
                            x_sb[:, kk, base + j : base + j + w_],
                            start=(kk == 0),
                            stop=(kk == 1),
                        )
                if wi == 2:
                    nc.scalar.copy(dst[:, base : base + wdt], pt[:, :wdt])
                    continue
                scr = sexp_pool.tile([128, N], F32, tag="sexp",
                                     name=f"sq_{wi}_{base}")
                # k: chunk the ACT copy at 768 so the DVE square+sum of each
                # chunk overlaps the copy of the next (spine shortening);
                # q: copy on DVE (single op), square+sum after.
                for sub in range(0, wdt, 768):
                    sw = min(768, wdt - sub)
                    lo, hi = base + sub, base + sub + sw
                    if wi == 1:
                        nc.scalar.copy(dst[:, lo:hi], pt[:, sub : sub + sw])
                    elif sub == 0:
                        nc.vector.tensor_copy(dst[:, base : base + wdt],
                                              pt[:, :wdt])
                    ssp = small_pool.tile([128, 1], F32, tag=f"ssp{ci}_{sub}",
                                          name=f"ssp_{wi}_{base}_{sub}")
                    nc.vector.scalar_tensor_tensor(
                        out=scr[:, lo:hi],
                        in0=dst[:, lo:hi],
                        scalar=1.0,
                        in1=dst[:, lo:hi],
                        op0=mybir.AluOpType.mult,
                        op1=mybir.AluOpType.mult,
                        accum_out=ssp[:],
                    )
                    ss_parts.setdefault(wi, []).append(ssp)

        # ---- v^T (+ ones row): [128 tokens-in-tile, (head, m-tile) x 65]
        vT = qkv_pool.tile([128, HEADS_PER_CORE * M_TILES * 65], F32R, name="vT")
        vT_v = vT.rearrange("p (t c) -> p t c", c=65)

        def emit_vT():
            nc.vector.tensor_copy(
                vT_v[:, :, 64:65],
                ones_col[:, 0:1].to_broadcast([128, HEADS_PER_CORE * M_TILES, 1]),
            )
            for j in range(HEADS_PER_CORE * M_TILES):
                h, t = divmod(j, M_TILES)
                pt = psum_av.tile([128, 512], F32, tag="av", name=f"tr_{j}")
                nc.tensor.matmul(
                    pt[:, :64],
                    v_sb[h * 64 : (h + 1) * 64, t * 128 : (t + 1) * 128],
                    ident_sb[h * 64 : (h + 1) * 64, h * 64 : (h + 1) * 64],
                    is_transpose=True,
                    start=True,
                    stop=True,
                )
                nc.vector.tensor_copy(vT_v[:, j, 0:64], pt[:, :64])

        emit_qkv(((0, q_sb), (1, k_sb)))
        emit_qkv(((2, v_sb),))

        # ---- l2 normalization: the normalizers 1/||q_d||, 1/||k_d|| are
        # per-(head,dim) ROW factors — the QK contraction dim — so their
        # product folds into a single per-partition scale on q; k stays raw.
        def combine(parts, tag, name):
            acc = parts[0]
            for i, p in enumerate(parts[1:]):
                nxt = small_pool.tile([128, 1], F32, tag=f"{tag}{i}",
                                      name=f"{name}{i}")
                nc.vector.tensor_add(nxt[:], acc[:], p[:])
                acc = nxt
            return acc

        ssq = combine(ss_parts[0], "ss", "ssq")
        ssk = combine(ss_parts[1], "nrm", "ssk")
        pp = small_pool.tile([128, 1], F32, tag="pp", name="pp")
        nc.vector.tensor_mul(pp[:], ssq[:], ssk[:])
        # g = rsqrt(ssq*ssk) via the quake bit-hack + 2 Newton iterations —
        # all on DVE, so no ACT table-set switch lands on the critical path.
        I32 = mybir.dt.int32
        magic = const_pool.tile([128, 1], I32, name="magic")
        nc.vector.memset(magic[:], 0x5F3759E0)  # 0x5f3759df + 1 (for ~t + 1)
        allones = const_pool.tile([128, 1], I32, name="allones")
        nc.vector.memset(allones[:], -1)
        sh1 = const_pool.tile([128, 1], I32, name="sh1")
        nc.vector.memset(sh1[:], 1)
        ti = small_pool.tile([128, 1], I32, tag="ip", name="ti")
        nc.vector.tensor_tensor(
            ti[:], pp[:].bitcast(I32), sh1[:], mybir.AluOpType.logical_shift_right
        )
        tn = small_pool.tile([128, 1], I32, tag="tn", name="tn")
        nc.vector.tensor_tensor(tn[:], ti[:], allones[:], mybir.AluOpType.bitwise_xor)
        y0 = small_pool.tile([128, 1], F32, tag="y0", name="y0")
        nc.vector.tensor_tensor(
            y0[:].bitcast(I32), tn[:], magic[:], mybir.AluOpType.add
        )
        # one Newton iteration: bit-hack seed err <=1.75e-3 -> ~4.6e-6,
        # far below the f32r rounding noise (~1e-4)
        yy = y0
        g = None
        for it in range(1):
            y2 = small_pool.tile([128, 1], F32, tag=f"y2_{it}", name=f"y2_{it}")
            nc.vector.tensor_mul(y2[:], yy[:], yy[:])
            tt = small_pool.tile([128, 1], F32, tag=f"tt_{it}", name=f"tt_{it}")
            nc.vector.tensor_mul(tt[:], y2[:], pp[:])
            sc = small_pool.tile([128, 1], F32, tag=f"sc_{it}", name=f"sc_{it}")
            nc.vector.tensor_scalar(
                out=sc[:], in0=tt[:], scalar1=-0.5, scalar2=1.5,
                op0=mybir.AluOpType.mult, op1=mybir.AluOpType.add,
            )
            g = small_pool.tile([128, 1], F32, tag=f"yn_{it}", name=f"yn_{it}")
            nc.vector.tensor_mul(g[:], yy[:], sc[:])
            yy = g
        # scale q in two chunks so the first QK block can start early
        nc.vector.tensor_scalar_mul(q_sb[:, 0:512], q_sb[:, 0:512], g[:])
        nc.vector.tensor_scalar_mul(q_sb[:, 512:N], q_sb[:, 512:N], g[:])

        # ---- attention + projection, software-pipelined over (block, head)
        out_sb = qkv_pool.tile([128, N], F32R, name="out_sb")
        y_sb = qkv_pool.tile([128, 2, N], F32, name="y_sb")
        yv = y_d.rearrange("(a p) n -> p a n", p=128)

        def emit_qk_exp(nb, w, h):
            """QK matmuls + exp for one (block, head); returns s_exp tile."""
            qh = q_sb[h * 64 : (h + 1) * 64]
            kh = k_sb[h * 64 : (h + 1) * 64]
            s_exp = sexp_pool.tile(
                [128, M_TILES * 512], F32R, tag="sexp", name=f"s_exp_{nb}_{h}"
            )
            for g in range(M_TILES // EXP_GROUP):
                pt = psum_s.tile([128, 1536], F32, tag="ps", name=f"qk_{nb}_{h}_{g}")
                for j in range(EXP_GROUP):
                    m = g * EXP_GROUP + j
                    nc.tensor.matmul(
                        pt[:, j * 512 : j * 512 + w],
                        kh[:, m * 128 : (m + 1) * 128],
                        qh[:, nb : nb + w],
                        start=True,
                        stop=True,
                    )
                o = s_exp[:, g * EXP_GROUP * w : (g + 1) * EXP_GROUP * w]
                if w == 512:
                    nc.scalar.activation(o, pt[:, : EXP_GROUP * 512], Exp)
                else:
                    i3 = pt.rearrange("p (b c) -> p b c", c=512)[:, :EXP_GROUP, :w]
                    o3 = o.rearrange("p (b c) -> p b c", c=w)
                    nc.scalar.activation(o3, i3, Exp)
            return s_exp

        def emit_av_divide(nb, w, h, s_exp, fast_tail=False):
            """AV accumulation + softmax divide for one (block, head)."""
            po = psum_av.tile([128, 512], F32, tag="av", name=f"av_{nb}_{h}")
            for m in range(M_TILES):
                nc.tensor.matmul(
                    po[:65, :w],
                    vT_v[:, h * M_TILES + m, :],
                    s_exp[:, m * w : (m + 1) * w],
                    start=(m == 0),
                    stop=(m == M_TILES - 1),
                )
            rd = small_pool.tile([1, 512], F32, tag="rd", name=f"rd_{nb}_{h}")
            nc.vector.reciprocal(rd[:, :w], po[64:65, :w])
            if fast_tail:
                # low-latency path: ones-matmul broadcast on the (idle) PE,
                # while ACT copies the unnormalized rows out of PSUM in
                # parallel; the final multiply then needs no serial bc copy.
                pbt = psum_av.tile([128, 512], F32, tag="av", name=f"pb_{nb}_{h}")
                nc.tensor.matmul(
                    pbt[:64, :w],
                    ones_sb[0:1, :].bitcast(F32),
                    rd[:1, :w],
                    start=True,
                    stop=True,
                )
                tmp = small_pool.tile([64, 512], F32, tag="bc", name=f"tm_{nb}_{h}")
                nc.scalar.copy(tmp[:, :w], po[0:64, :w])
                nc.vector.tensor_mul(
                    out_sb[h * 64 : (h + 1) * 64, nb : nb + w],
                    pbt[0:64, :w],
                    tmp[:, :w],
                )
                return
            # partition-broadcast via a DMA round-trip through DRAM
            bc = small_pool.tile([64, 512], F32, tag="bc", name=f"bc_{nb}_{h}")
            scr_d = dram_pool.tile([1, 512], F32, tag="dscr",
                                   name=f"dscr_{nb}_{h}")
            nc.sync.dma_start(scr_d[:, :w], rd[:, :w])
            nc.sync.dma_start(bc[:, :w], scr_d[:1, :w].to_broadcast([64, w]))
            nc.vector.tensor_mul(
                out_sb[h * 64 : (h + 1) * 64, nb : nb + w],
                po[0:64, :w],
                bc[:, :w],
            )

        def emit_proj(nb, w):
            """Output projection + bias + store for one token block. The two
            psum tiles come from the AV pool so the QK pool stays a pure
            rotation (a proj tile in the QK rotation shifts the next block's
            first QK group onto the exp critical path)."""
            for m2 in range(2):
                pj = psum_av.tile([128, 512], F32, tag="av", name=f"proj_{nb}_{m2}")
                nc.tensor.matmul(
                    pj[:, :w],
                    wp_sb[:, m2 * 128 : (m2 + 1) * 128],
                    out_sb[:, nb : nb + w],
                    start=True,
                    stop=True,
                )
                nc.vector.tensor_scalar_add(
                    y_sb[:, m2, nb : nb + w],
                    pj[:, :w],
                    bias_sb[:, m2 : m2 + 1],
                )
            nc.sync.dma_start(yv[:, :, nb : nb + w], y_sb[:, :, nb : nb + w])

        emit_vT()

        items = [(nb, w, h) for (nb, w) in BLOCKS for h in range(HEADS_PER_CORE)]
        s_tiles = {}
        for idx, it in enumerate(items):
            s_tiles[idx] = emit_qk_exp(*it)
            if idx >= 1:
                pit = items[idx - 1]
                emit_av_divide(*pit, s_tiles.pop(idx - 1))
            if idx >= 2 and items[idx - 2][2] == HEADS_PER_CORE - 1:
                emit_proj(items[idx - 2][0], items[idx - 2][1])
        emit_av_divide(*items[-1], s_tiles.pop(len(items) - 1), fast_tail=True)
        if items[-2][2] == HEADS_PER_CORE - 1:
            emit_proj(items[-2][0], items[-2][1])
        emit_proj(items[-1][0], items[-1][1])


def _get_nc():
    if "nc" not in _CACHE:
        _CACHE["nc"] = _build_kernel()
    return _CACHE["nc"]


def _round_f32r(a):
    """Round fp32 to fp32r (TF32-like: 11-bit mantissa, round-half-up on
    magnitude). The on-device DVE staging copies also round, but rounding on
    the host keeps host and device data bit-identical."""
    u = np.ascontiguousarray(a, dtype=np.float32).view(np.uint32)
    r = ((u.astype(np.uint64) + 0x800) & 0xFFFFF000).astype(np.uint32)
    return r.view(np.float32)


def _make_in_maps(x, w_qkv, w_proj, b_proj):
    x = np.ascontiguousarray(np.asarray(x, dtype=np.float32)).reshape(B, C, N)
    w_qkv = np.asarray(w_qkv, dtype=np.float32)
    w_proj = np.asarray(w_proj, dtype=np.float32)
    b_proj = np.asarray(b_proj, dtype=np.float32)
    ident = np.eye(128, dtype=np.float32)

    in_maps = []
    for core in range(N_CORES):
        b = core // 4
        hg = core % 4
        r = 128 * hg
        wq = np.ascontiguousarray(w_qkv[r : r + 128, :].T)  # [C, 128]
        wk = np.ascontiguousarray(w_qkv[512 + r : 512 + r + 128, :].T)
        wv = np.ascontiguousarray(w_qkv[1024 + r : 1024 + r + 128, :].T)
        wp = np.ascontiguousarray(w_proj[:, r : r + 128].T)  # [128, C]
        bias = (
            b_proj.reshape(C, 1)
            if hg == 0
            else np.zeros((C, 1), dtype=np.float32)
        )
        in_maps.append(
            {
                "x": _round_f32r(x[b]),
                "wq": _round_f32r(wq),
                "wk": _round_f32r(wk),
                "wv": _round_f32r(wv),
                "wp": _round_f32r(wp),
                "ident": ident,
                "ones": np.ones((128, 64), dtype=np.float32),
                "bias": np.ascontiguousarray(bias),
            }
        )
    return in_maps


def run_spmd(x, w_qkv, w_proj, b_proj, trace=False):
    """Run the SPMD kernel on cores 0-7; returns (y, BassKernelResults)."""
    nc = _get_nc()
    in_maps = _make_in_maps(x, w_qkv, w_proj, b_proj)
    res = bass_utils.run_bass_kernel_spmd(
        nc, in_maps, core_ids=list(range(N_CORES)), trace=trace
    )
    y = np.zeros((B, C, N), dtype=np.float32)
    for core in range(N_CORES):
        y[core // 4] += res.results[core]["y"]
    return y.reshape(B, C, 48, 48), res


def kernel(x, w_qkv, w_proj, b_proj):
    y, _ = run_spmd(x, w_qkv, w_proj, b_proj, trace=False)
    return y

